# revision 1
# baseline (speedup 1.0000x reference)
"""Binary-conv BasicBlock (sign-act 3x3 binary conv + BN(eval) + residual).

Full shapes: x (32,128,56,56) f32, weight (128,128,3,3), BN params (128,).
Strategy: data-parallel over batch N across 8 NeuronCores (4 images/core).
Per image on-device:
  - sign(x) computed on ScalarE into a zero-padded bf16 tile [128, 58, 58]
  - conv = 9 shifted matmuls (taps) accumulating in PSUM; lhsT = sign(W) taps
    as [C=128, P=128] bf16.  +/-1 values are exact in bf16; integer partial
    sums (<=1152) are exact in fp32 PSUM -> conv is bit-exact.
  - BN folded to out = conv * s + t on ScalarE (PSUM -> SBUF)
  - residual add x on VectorE, DMA out.
"""

import numpy as np
import ml_dtypes

_N, _C, _H, _W = 32, 128, 56, 56
_P = 128
_NCORES = 8
_NPI = _N // _NCORES  # images per core
_HP, _WP = _H + 2, _W + 2
_NPIX = _H * _W
_BN_EPS = 1e-5
_CH = 8               # output rows per PSUM chunk (8*56=448 <= 512 f32/bank)
_NCH = _H // _CH      # 7 chunks per image

_cache = {}


def _build_program():
    import concourse.bacc as bacc
    import concourse.mybir as mybir
    import concourse.tile as tile

    f32 = mybir.dt.float32
    bf16 = mybir.dt.bfloat16

    nc = bacc.Bacc("TRN2", target_bir_lowering=False, debug=False)

    x_d = nc.dram_tensor("x", [_NPI, _C, _NPIX], f32, kind="ExternalInput")
    w_d = nc.dram_tensor("w", [_C, 9, _P], bf16, kind="ExternalInput")
    s_d = nc.dram_tensor("s", [_P, 1], f32, kind="ExternalInput")
    t_d = nc.dram_tensor("t", [_P, 1], f32, kind="ExternalInput")
    o_d = nc.dram_tensor("o", [_NPI, _P, _NPIX], f32, kind="ExternalOutput")

    SIGN = mybir.ActivationFunctionType.Sign
    IDENT = mybir.ActivationFunctionType.Identity

    with tile.TileContext(nc) as tc:
        with (
            tc.tile_pool(name="const", bufs=1) as cpool,
            tc.tile_pool(name="xin", bufs=3) as xpool,
            tc.tile_pool(name="apad", bufs=1) as apool,
            tc.tile_pool(name="outp", bufs=2) as opool,
            tc.tile_pool(name="ps", bufs=8, space="PSUM") as pspool,
        ):
            wt = cpool.tile([_C, 9, _P], bf16)
            nc.sync.dma_start(wt[:], w_d[:])
            s_t = cpool.tile([_P, 1], f32)
            nc.sync.dma_start(s_t[:], s_d[:])
            t_t = cpool.tile([_P, 1], f32)
            nc.sync.dma_start(t_t[:], t_d[:])

            # Two persistent padded sign tiles; borders zeroed once, only the
            # interior is rewritten per image.
            a_tiles = []
            for i in range(2):
                a_t = apool.tile([_C, _HP, _WP], bf16, name=f"apad{i}", tag=f"apad{i}")
                nc.any.memset(a_t[:], 0.0)
                a_tiles.append(a_t)

            x_tiles = [None] * _NPI

            def load_and_sign(n):
                x_t = xpool.tile([_C, _NPIX], f32, name="x_t", tag="x")
                nc.sync.dma_start(x_t[:], x_d[n])
                x_tiles[n] = x_t
                a_t = a_tiles[n % 2]
                nc.scalar.activation(
                    a_t[:, 1 : _H + 1, 1 : _W + 1],
                    x_t[:].rearrange("c (h w) -> c h w", h=_H),
                    SIGN,
                )

            load_and_sign(0)
            for n in range(_NPI):
                # Emit next image's sign ahead of this image's epilogue so the
                # in-order ScalarE stream never stalls the next image's matmuls.
                if n + 1 < _NPI:
                    load_and_sign(n + 1)
                x_t = x_tiles[n]
                a_t = a_tiles[n % 2]
                out_t = opool.tile([_P, _NPIX], f32, name="out_t", tag="o")

                psums = [
                    pspool.tile([_P, 512], f32, name=f"ps{c}", tag="ps")
                    for c in range(_NCH)
                ]
                for tp in range(9):
                    kh, kw = tp // 3, tp % 3
                    for c in range(_NCH):
                        r0 = c * _CH
                        nc.tensor.matmul(
                            psums[c][:, : _CH * _W],
                            wt[:, tp, :],
                            a_t[:, r0 + kh : r0 + kh + _CH, kw : kw + _W],
                            start=(tp == 0),
                            stop=(tp == 8),
                        )
                for c in range(_NCH):
                    sl = slice(c * _CH * _W, (c + 1) * _CH * _W)
                    nc.scalar.activation(
                        out_t[:, sl],
                        psums[c][:, : _CH * _W],
                        IDENT,
                        bias=t_t[:, 0:1],
                        scale=s_t[:, 0:1],
                    )
                    nc.vector.tensor_add(out_t[:, sl], out_t[:, sl], x_t[:, sl])
                nc.sync.dma_start(o_d[n], out_t[:])

    nc.compile()
    return nc


def _get_program():
    if "nc" not in _cache:
        _cache["nc"] = _build_program()
    return _cache["nc"]


def _prep_inputs(x, weight, bias, gamma, beta, running_mean, running_var):
    # per-core batch shards
    xs = np.ascontiguousarray(
        np.asarray(x, dtype=np.float32).reshape(_NCORES, _NPI, _C, _NPIX)
    )
    # sign(weight) as [C, tap, P] bf16 (lhsT per tap)
    wb = np.sign(np.asarray(weight, dtype=np.float32))  # [P, C, 3, 3]
    wT = np.ascontiguousarray(
        wb.transpose(1, 2, 3, 0).reshape(_C, 9, _P)
    ).astype(ml_dtypes.bfloat16)
    inv = np.asarray(gamma, dtype=np.float64) / np.sqrt(
        np.asarray(running_var, dtype=np.float64) + _BN_EPS
    )
    shift = (
        np.asarray(bias, dtype=np.float64) * inv
        + np.asarray(beta, dtype=np.float64)
        - np.asarray(running_mean, dtype=np.float64) * inv
    )
    s = inv.astype(np.float32).reshape(_P, 1)
    t = shift.astype(np.float32).reshape(_P, 1)
    return [
        {"x": xs[i], "w": wT, "s": s, "t": t} for i in range(_NCORES)
    ]


def _run(inputs, trace=False, trace_cores=None):
    from concourse.bass_utils import run_bass_kernel_spmd

    nc = _get_program()
    in_maps = _prep_inputs(**inputs)
    res = run_bass_kernel_spmd(
        nc,
        in_maps,
        list(range(_NCORES)),
        trace=trace,
        trace_cores=trace_cores,
    )
    out = np.stack([res.results[i]["o"] for i in range(_NCORES)], axis=0)
    out = out.reshape(_N, _P, _H, _W).astype(np.float32, copy=False)
    return out, res


def kernel(**inputs):
    out, _ = _run(inputs, trace=False)
    return out


# revision 2
# speedup vs baseline: 1.1390x; 1.1390x over previous
"""Binary-conv BasicBlock (sign-act 3x3 binary conv + BN(eval) + residual).

Full shapes: x (32,128,56,56) f32, weight (128,128,3,3), BN params (128,).
Strategy: data-parallel over batch N across 8 NeuronCores (4 images/core).
Per image on-device:
  - sign(x) computed on ScalarE into a zero-padded bf16 tile [128, 58, 58]
  - conv = 9 shifted matmuls (taps) accumulating in PSUM; lhsT = sign(W) taps
    as [C=128, P=128] bf16.  +/-1 values are exact in bf16; integer partial
    sums (<=1152) are exact in fp32 PSUM -> conv is bit-exact.
  - chunk-major: 7 output rows (392 px) per PSUM bank, 9-tap groups per bank,
    2 banks per "pair"; per-pair epilogue (ScalarE: conv*s + t from PSUM) +
    residual add (VectorE) + per-pair store overlap the next pair's matmuls.
  - x loads split in halves w/ split sign ops for a fast pipeline start;
    warmup matmuls keep the PE HAM un-throttled through the initial DMA wait.
"""

import numpy as np
import ml_dtypes

_N, _C, _H, _W = 32, 128, 56, 56
_P = 128
_NCORES = 8
_NPI = _N // _NCORES  # images per core
_HP, _WP = _H + 2, _W + 2
_NPIX = _H * _W
_BN_EPS = 1e-5
_CH = 7               # output rows per PSUM bank chunk (7*56=392 <= 512)
_NCH = _H // _CH      # 8 chunks per image
_NPAIR = _NCH // 2    # 4 pair-tiles (2 banks each) per image
_CN = _CH * _W        # 392 elems per chunk

_cache = {}


def _build_program():
    import concourse.bacc as bacc
    import concourse.mybir as mybir
    import concourse.tile as tile

    f32 = mybir.dt.float32
    bf16 = mybir.dt.bfloat16

    nc = bacc.Bacc("TRN2", target_bir_lowering=False, debug=False)

    x_d = nc.dram_tensor("x", [_NPI, _C, _NPIX], f32, kind="ExternalInput")
    w_d = nc.dram_tensor("w", [_C, 9, _P], bf16, kind="ExternalInput")
    s_d = nc.dram_tensor("s", [_P, 1], f32, kind="ExternalInput")
    t_d = nc.dram_tensor("t", [_P, 1], f32, kind="ExternalInput")
    o_d = nc.dram_tensor("o", [_NPI, _P, _NPIX], f32, kind="ExternalOutput")

    SIGN = mybir.ActivationFunctionType.Sign
    IDENT = mybir.ActivationFunctionType.Identity
    HROWS = _H // 2  # 28 rows per x half

    with tile.TileContext(nc) as tc:
        with (
            tc.tile_pool(name="const", bufs=1) as cpool,
            tc.tile_pool(name="xin", bufs=3) as xpool,
            tc.tile_pool(name="apad", bufs=1) as apool,
            tc.tile_pool(name="outp", bufs=4) as opool,
            tc.tile_pool(name="ps", bufs=4, space="PSUM") as pspool,
        ):
            # Warmup source: tiny zero tile; matmuls on it keep the PE busy
            # (HAM stays at 8/8) while the first image loads.
            dummy = cpool.tile([_C, _P], bf16)
            nc.any.memset(dummy[:], 0.0)

            # x halves issued before the small param DMAs: x0 is on the
            # startup critical path.
            x_tiles = [None] * _NPI

            def load_x(n):
                x_t = xpool.tile([_C, _NPIX], f32, name="x_t", tag="x")
                for h in range(2):
                    nc.sync.dma_start(
                        x_t[:, h * HROWS * _W : (h + 1) * HROWS * _W],
                        x_d[n, :, h * HROWS * _W : (h + 1) * HROWS * _W],
                    )
                x_tiles[n] = x_t
                return x_t

            load_x(0)

            wt = cpool.tile([_C, 9, _P], bf16)
            nc.sync.dma_start(wt[:], w_d[:])
            s_t = cpool.tile([_P, 1], f32)
            nc.sync.dma_start(s_t[:], s_d[:])
            t_t = cpool.tile([_P, 1], f32)
            nc.sync.dma_start(t_t[:], t_d[:])

            # Two persistent padded sign tiles; borders zeroed once, only the
            # interior is rewritten per image.
            a_tiles = []
            for i in range(2):
                a_t = apool.tile([_C, _HP, _WP], bf16, name=f"apad{i}", tag=f"apad{i}")
                nc.any.memset(a_t[:], 0.0)
                a_tiles.append(a_t)

            def sign_img(n):
                x_v = x_tiles[n][:].rearrange("c (h w) -> c h w", h=_H)
                a_t = a_tiles[n % 2]
                for h in range(2):
                    r = h * HROWS
                    nc.scalar.activation(
                        a_t[:, 1 + r : 1 + r + HROWS, 1 : _W + 1],
                        x_v[:, r : r + HROWS, :],
                        SIGN,
                    )

            sign_img(0)

            # PE warmup: ~40 cheap matmuls into the first psum slots while
            # DMA+sign of image 0 are in flight (start/stop=True, results
            # discarded when the real accumulation group restarts the bank).
            warm_ps = pspool.tile([_P, 2, 512], f32, name="warm_ps", tag="ps")
            for i in range(40):
                nc.tensor.matmul(
                    warm_ps[:, i % 2, :128],
                    dummy[:],
                    dummy[:],
                    start=True,
                    stop=True,
                )

            for n in range(_NPI):
                # Emit next image's load+sign ahead of this image's epilogue
                # so the in-order ScalarE stream never stalls next matmuls.
                if n + 1 < _NPI:
                    load_x(n + 1)
                    sign_img(n + 1)
                x_t = x_tiles[n]
                a_t = a_tiles[n % 2]

                for p in range(_NPAIR):
                    pst = pspool.tile([_P, 2, 512], f32, name="pst", tag="ps")
                    for b in range(2):
                        c = 2 * p + b
                        r0 = c * _CH
                        for tp in range(9):
                            kh, kw = tp // 3, tp % 3
                            nc.tensor.matmul(
                                pst[:, b, :_CN],
                                wt[:, tp, :],
                                a_t[:, r0 + kh : r0 + kh + _CH, kw : kw + _W],
                                start=(tp == 0),
                                stop=(tp == 8),
                            )
                    sl = slice(p * 2 * _CN, (p + 1) * 2 * _CN)
                    out_t = opool.tile([_P, 2 * _CN], f32, name="out_t", tag="o")
                    nc.scalar.activation(
                        out_t[:],
                        pst[:, :, :_CN],
                        IDENT,
                        bias=t_t[:, 0:1],
                        scale=s_t[:, 0:1],
                    )
                    nc.vector.tensor_add(out_t[:], out_t[:], x_t[:, sl])
                    nc.sync.dma_start(o_d[n, :, sl], out_t[:])

    nc.compile()
    return nc


def _get_program():
    if "nc" not in _cache:
        _cache["nc"] = _build_program()
    return _cache["nc"]


def _prep_inputs(x, weight, bias, gamma, beta, running_mean, running_var):
    # per-core batch shards
    xs = np.ascontiguousarray(
        np.asarray(x, dtype=np.float32).reshape(_NCORES, _NPI, _C, _NPIX)
    )
    # sign(weight) as [C, tap, P] bf16 (lhsT per tap)
    wb = np.sign(np.asarray(weight, dtype=np.float32))  # [P, C, 3, 3]
    wT = np.ascontiguousarray(
        wb.transpose(1, 2, 3, 0).reshape(_C, 9, _P)
    ).astype(ml_dtypes.bfloat16)
    inv = np.asarray(gamma, dtype=np.float64) / np.sqrt(
        np.asarray(running_var, dtype=np.float64) + _BN_EPS
    )
    shift = (
        np.asarray(bias, dtype=np.float64) * inv
        + np.asarray(beta, dtype=np.float64)
        - np.asarray(running_mean, dtype=np.float64) * inv
    )
    s = inv.astype(np.float32).reshape(_P, 1)
    t = shift.astype(np.float32).reshape(_P, 1)
    return [
        {"x": xs[i], "w": wT, "s": s, "t": t} for i in range(_NCORES)
    ]


def _run(inputs, trace=False, trace_cores=None):
    from concourse.bass_utils import run_bass_kernel_spmd

    nc = _get_program()
    in_maps = _prep_inputs(**inputs)
    res = run_bass_kernel_spmd(
        nc,
        in_maps,
        list(range(_NCORES)),
        trace=trace,
        trace_cores=trace_cores,
    )
    out = np.stack([res.results[i]["o"] for i in range(_NCORES)], axis=0)
    out = out.reshape(_N, _P, _H, _W).astype(np.float32, copy=False)
    return out, res


def kernel(**inputs):
    out, _ = _run(inputs, trace=False)
    return out
